# revision 18
# baseline (speedup 1.0000x reference)
"""Trainium2 Bass kernel for nn_ComposerModule (dense_transformer), v5.

Data-parallel over batch: 32 batch items -> 8 NeuronCores, 4 per core.

The four per-core batch items are processed TOGETHER in 32-partition strips
of [128, S] tiles (batch b owns partitions 32b..32b+15; rows 32b+16..32b+31
are zero pads).  Per-batch [O=16, S] softmax ops become single [128, S]
ops; thin-M matmuls run 4-way concurrent via tile_position tiling.

The residual stream is kept ONLY in xn ([s, h]) orientation.  Logits are
accumulated INCREMENTALLY in f32 PSUM instead of re-projecting x:
    lg_{l+1} = lg_l + G @ opwt,   G = t @ (Wv^T oqk) + ws x (bv oqk)
xt is materialized only at layer 0 (PE transposes of the gathered
embedding, overlapping the SWDGE gathers) and after the last layer (xbar
transposes feeding the final projection).

v5 vs v4 (both profiled on HW): the embedding PSUM pool is scoped so the
layer loop gets 5 free banks; the out+residual stage is ONE K=32 N=1024
bf16-PSUM matmul per (batch, s-chunk) -- 4-way row-tile concurrent -- and
ONE [128,1024] bf16 2x-mode DVE add (batch 3 goes ACT copy + GPSIMD add
for engine balance); t^T runs on the PE (identity transposes) so the PE
has no idle window there (HAM stays warm).

Algebra: v-projection folded, both softmaxes share one exp:
  w[o,s]  = e[o,s]/rowsum * ops[o,s];  t = w @ x;  oo = t @ Wv^T + ws*bv
  out[s,h] = sum_o e[o,s]/colsum[s] * oo[o,h];  x += out
Pad hygiene: oqkt pad cols are 0 and c pad rows are -30, so
e_pad = exp(-30) ~ 1e-13; ops_strip/A2/g0 pad entries are 0 so
w/ws/t/oo/G/delta-lg pads are exactly 0.
"""
import math

import numpy as np
import ml_dtypes

B, S, H, O, V, OUT, L = 32, 512, 1024, 16, 32000, 1000, 4
NCORES = 8
BPC = B // NCORES
BF16 = ml_dtypes.bfloat16

# packed-weights column offsets (bf16 [128, WC])
_PEN0 = 0              # pe chunked [128, 4*1024]
_BVB0 = 4096           # bv tiled   [128, 1024]
_OPS0 = 5120           # ops strips [128, 512]
_OQK0 = 5632           # oqkT pad   [128, 8*32]
_IDN0 = 5888           # identity   [128, 128]
_BD0 = 6016            # block-diag [128, 128]
_A20 = 6144            # Wv^T@oqkT pad [128, 8*32]
_G00 = 6400            # bv@oqkT pad   [128, 32]
WC = 6432

_cache = {}


def _sinusoidal_pos_emb(seq_len, dim):
    pos = np.arange(seq_len)[:, None].astype(np.float32)
    div = np.exp(np.arange(0, dim, 2).astype(np.float32) * (-math.log(10000.0) / dim))
    pe = np.zeros((seq_len, dim), dtype=np.float32)
    pe[:, 0::2] = np.sin(pos * div)
    pe[:, 1::2] = np.cos(pos * div)
    return pe


def _build_program():
    import concourse.bacc as bacc
    import concourse.bass as bass
    import concourse.tile as tile
    from concourse import mybir

    dt = mybir.dt
    f32, bf16, i16 = dt.float32, dt.bfloat16, dt.int16
    PSUM = bass.MemorySpace.PSUM
    Alu = mybir.AluOpType
    Act = mybir.ActivationFunctionType

    nc = bacc.Bacc("TRN2", target_bir_lowering=False, debug=False, num_devices=NCORES)

    emb_d = nc.declare_dram_parameter("emb", [V, H], bf16, isOutput=False)
    tok_d = nc.declare_dram_parameter("tok", [128, BPC, S // 16], i16, isOutput=False)
    wpk_d = nc.declare_dram_parameter("wpk", [128, WC], bf16, isOutput=False)
    cst_d = nc.declare_dram_parameter("cst", [128, 1], f32, isOutput=False)
    wvt_d = nc.declare_dram_parameter("wvt", [128, 8, H], bf16, isOutput=False)
    wot_d = nc.declare_dram_parameter("wot", [128, 8, OUT], bf16, isOutput=False)
    out_d = nc.declare_dram_parameter("out", [BPC, 4, 128, OUT], bf16, isOutput=True)

    with tile.TileContext(nc) as tc:
        with (
            tc.tile_pool(name="wts", bufs=1) as wp,
            tc.tile_pool(name="xres", bufs=1) as xp,
            tc.tile_pool(name="work", bufs=2) as wk,
            tc.tile_pool(name="sm", bufs=2) as sm,
            tc.tile_pool(name="psG", bufs=1, space=PSUM) as psG,
            tc.tile_pool(name="psW", bufs=2, space=PSUM) as psW,
        ):
            # ---- persistent weights
            wpk = wp.tile([128, WC], bf16)
            c_sb = wp.tile([128, 1], f32)
            wvt = wp.tile([128, 8, H], bf16)
            wot = wp.tile([128, 8, OUT], bf16)
            tokt = wp.tile([128, BPC, S // 16], i16)

            def pen(cc):
                return wpk[:, _PEN0 + cc * H:_PEN0 + (cc + 1) * H]

            def bvb(n):
                return wpk[:, _BVB0 + n * 512:_BVB0 + (n + 1) * 512]

            ops_s = wpk[:, _OPS0:_OPS0 + 512]

            def oqkt(k):
                return wpk[:, _OQK0 + k * 32:_OQK0 + (k + 1) * 32]

            idn = wpk[:, _IDN0:_IDN0 + 128]
            bd = wpk[:, _BD0:_BD0 + 128]

            def a2p(k):
                return wpk[:, _A20 + k * 32:_A20 + (k + 1) * 32]

            g0b = wpk[:, _G00:_G00 + 32]

            # startup loads: tok + packed weights on sync, wvt/wot on scalar
            nc.sync.dma_start(tokt[:], tok_d[:])
            nc.sync.dma_start(wpk[:], wpk_d[:])
            nc.sync.dma_start(c_sb[:], cst_d[:])
            nc.scalar.dma_start(wvt[:, 0:4], wvt_d[:, 0:4])
            nc.scalar.dma_start(wvt[:, 4:8], wvt_d[:, 4:8])

            # ---- residual stream (bf16): xn master; xt only at entry/exit
            xt = [xp.tile([128, 8, S], bf16, name=f"xt{b}") for b in range(BPC)]
            xn = [xp.tile([128, 4, H], bf16, name=f"xn{b}") for b in range(BPC)]

            # persistent f32 logits accumulator [strip, s]
            lg = psG.tile([128, S], f32, name="lg")

            # ---- embedding: SWDGE gather -> xn; +pe; PE-transpose -> xt0;
            # layer-0 logits emitted per batch (keeps the PE FIFO flowing)
            with tc.tile_pool(name="psT", bufs=2, space=PSUM) as psT:
                for b in range(BPC):
                    for hh in range(2):
                        nc.gpsimd.dma_gather(
                            out_ap=xn[b][:, hh * 2:(hh + 1) * 2, :],
                            in_ap=emb_d[:],
                            idxs_ap=tokt[:, b, hh * 16:(hh + 1) * 16],
                            num_idxs=S // 2, num_idxs_reg=S // 2, elem_size=H,
                            transpose=False)
                for b in range(BPC):
                    for cc in range(4):
                        nc.vector.tensor_tensor(xn[b][:, cc, :],
                                                xn[b][:, cc, :],
                                                pen(cc), op=Alu.add)
                    for k in range(8):
                        ttp = psT.tile([128, 4, 128], bf16, tag="tr", bufs=2,
                                       name=f"ept_{b}_{k}")
                        for cc in range(4):
                            nc.tensor.transpose(
                                ttp[:, cc, :],
                                xn[b][:, cc, k * 128:(k + 1) * 128], idn)
                        if k % 2 == 0:
                            nc.vector.tensor_copy(xt[b][:, k, :], ttp[:])
                        else:
                            nc.scalar.copy(xt[b][:, k, :], ttp[:])
                    for k in range(8):
                        nc.tensor.matmul(lg[32 * b:32 * (b + 1), :],
                                         oqkt(k), xt[b][:, k, :],
                                         start=(k == 0), stop=False,
                                         tile_position=(0, 32 * b),
                                         skip_group_check=True)

            # final-projection weights (needed only after the last layer)
            nc.scalar.dma_start(wot[:, 0:4], wot_d[:, 0:4])
            nc.scalar.dma_start(wot[:, 4:8], wot_d[:, 4:8])

            with tc.tile_pool(name="psO", bufs=5, space=PSUM) as psO:

                def emit_final(b):
                    # out[s, :] = x @ Wout^T  (bout added on host)
                    for cc in range(4):
                        nc.sync.dma_start(
                            xt[b][:, :, cc * 128:(cc + 1) * 128],
                            xn[b][:, cc, :], transpose=True)
                    for mg in range(4):
                        osb = wk.tile([128, OUT], bf16, tag="osb", bufs=3)
                        pA = psO.tile([128, 512], f32, tag="rs", bufs=5,
                                      name=f"fA_{b}_{mg}")
                        pB = psO.tile([128, 512], f32, tag="rs", bufs=5,
                                      name=f"fB_{b}_{mg}")
                        for k in range(8):
                            nc.tensor.matmul(
                                pA[:], xt[b][:, k, mg * 128:(mg + 1) * 128],
                                wot[:, k, 0:512], start=(k == 0),
                                stop=(k == 7))
                            nc.tensor.matmul(
                                pB[:, 0:OUT - 512],
                                xt[b][:, k, mg * 128:(mg + 1) * 128],
                                wot[:, k, 512:OUT], start=(k == 0),
                                stop=(k == 7))
                        nc.vector.tensor_copy(osb[:, 0:512], pA[:])
                        nc.scalar.copy(osb[:, 512:OUT], pB[:, 0:OUT - 512])
                        nc.sync.dma_start(out_d[b, mg], osb[:])

                def emit_out_resid(l, b, cc):
                    # n=0 half: plain matmul, drained by a DVE psum add
                    po = psO.tile([128, 512], f32, tag="rs", bufs=5,
                                  name=f"o_{l}_{cc}_{b}")
                    nc.tensor.matmul(
                        po[:],
                        opwt_cur[32 * b:32 * (b + 1),
                                 cc * 128:(cc + 1) * 128],
                        oo_cur[32 * b:32 * (b + 1), 0:512],
                        start=True, stop=True, tile_position=(32 * b, 0))
                    # n=1 half: b 0/1 plain (ACT copy + GPSIMD add), b 2/3
                    # PE identity-accumulate (ACT copy)
                    q = psO.tile([128, 512], f32, tag="rs", bufs=5,
                                 name=f"q_{l}_{cc}_{b}")
                    if b >= 2:
                        nc.tensor.matmul(q[:], idn, xn[b][:, cc, 512:1024],
                                         start=True, stop=False)
                    nc.tensor.matmul(
                        q[:],
                        opwt_cur[32 * b:32 * (b + 1),
                                 cc * 128:(cc + 1) * 128],
                        oo_cur[32 * b:32 * (b + 1), 512:1024],
                        start=(b < 2), stop=True,
                        skip_group_check=True, tile_position=(32 * b, 0))
                    return po, q

                def emit_add(b, cc, poq, l):
                    po, q = poq
                    nc.vector.tensor_tensor(xn[b][:, cc, 0:512],
                                            xn[b][:, cc, 0:512],
                                            po[:], op=Alu.add)
                    if b < 2:
                        rtmp = sm.tile([128, 512], bf16, tag="rtmp", bufs=2,
                                       name=f"rt_{l}_{cc}_{b}")
                        nc.scalar.copy(rtmp[:], q[:])
                        nc.gpsimd.tensor_tensor(xn[b][:, cc, 512:1024],
                                                xn[b][:, cc, 512:1024],
                                                rtmp[:], op=Alu.add)
                    else:
                        nc.scalar.copy(xn[b][:, cc, 512:1024], q[:])

                # ---- layer stages (stage-major emission, all batches)
                def emit_front(l):
                    """S2-S4a: exp, colsum/reciprocals, w, opwt, w^T DMA.
                    Depends only on lg (deltas from layer l-1), not on xn."""
                    e_all = sm.tile([128, S], bf16, tag="e", bufs=2,
                                    name=f"e_{l}")
                    rs = sm.tile([128, 1], f32, tag="rs", bufs=2)
                    nc.scalar.activation(e_all[:], lg[:], Act.Exp,
                                         bias=c_sb[:], accum_out=rs[:])

                    cs_ps = psW.tile([128, S], f32, tag="tw", name=f"cs_{l}")
                    nc.tensor.matmul(cs_ps[:], bd[:], e_all[:], start=True,
                                     stop=True)
                    rcb = sm.tile([128, S], f32, tag="rcb", bufs=2,
                                  name=f"rcb_{l}")
                    nc.vector.reciprocal_approx_fast(rcb[:], cs_ps[:])

                    rcs = sm.tile([128, 1], f32, tag="rcs", bufs=2)
                    nc.vector.reciprocal(rcs[:], rs[:])
                    w_all = sm.tile([128, S], bf16, tag="w", bufs=2,
                                    name=f"w_{l}")
                    ws = sm.tile([128, 1], f32, tag="ws", bufs=2)
                    nc.vector.scalar_tensor_tensor(w_all[:], e_all[:],
                                                   rcs[:], ops_s,
                                                   op0=Alu.mult,
                                                   op1=Alu.mult,
                                                   accum_out=ws[:])

                    opwt = sm.tile([128, S], bf16, tag="opwt", bufs=2,
                                   name=f"opwt_{l}")
                    nc.vector.tensor_tensor(opwt[:], e_all[:], rcb[:],
                                            op=Alu.mult)

                    wt_sb = sm.tile([128, 4, 128], bf16, tag="wt", bufs=2,
                                    name=f"wt_{l}")
                    nc.sync.dma_start(wt_sb[:], w_all[:], transpose=True)
                    return {"opwt": opwt, "ws": ws, "wt_sb": wt_sb}

                def emit_mid(l, st):
                    """S4b-S5b: t, t^T, oo, and the lg delta for layer l+1.
                    Needs xn (layer l-1's residual complete)."""
                    wt_sb, ws, opwt = st["wt_sb"], st["ws"], st["opwt"]
                    t_ps = [psW.tile([128, 512], f32, tag="tw",
                                     name=f"t_{l}_{n}") for n in range(2)]
                    for cc in range(4):
                        for n in range(2):
                            for b in range(BPC):
                                nc.tensor.matmul(
                                    t_ps[n][32 * b:32 * (b + 1), :],
                                    wt_sb[:, cc, 32 * b:32 * (b + 1)],
                                    xn[b][:, cc, n * 512:(n + 1) * 512],
                                    start=(cc == 0), stop=(cc == 3),
                                    tile_position=(0, 32 * b))
                    t_sb = sm.tile([128, H], bf16, tag="tsb", bufs=2,
                                   name=f"t_{l}")
                    nc.scalar.copy(t_sb[:, 0:512], t_ps[0][:])
                    nc.scalar.copy(t_sb[:, 512:], t_ps[1][:])

                    tt_sb = sm.tile([128, 8, 128], bf16, tag="tt", bufs=2,
                                    name=f"tt_{l}")
                    for g in range(2):
                        trp = psW.tile([128, 4, 128], bf16, tag="tw",
                                       name=f"tr_{l}_{g}")
                        for k in range(4):
                            nc.tensor.transpose(
                                trp[:, k, :],
                                t_sb[:, (g * 4 + k) * 128:
                                     (g * 4 + k + 1) * 128], idn)
                        nc.vector.tensor_copy(tt_sb[:, g * 4:(g + 1) * 4],
                                              trp[:])
                    oo_ps = [psW.tile([128, 512], f32, tag="tw",
                                      name=f"oo_{l}_{n}") for n in range(2)]
                    for n in range(2):
                        for k in range(8):
                            nc.tensor.matmul(oo_ps[n][:], tt_sb[:, k, :],
                                             wvt[:, k,
                                                 n * 512:(n + 1) * 512],
                                             start=(k == 0), stop=(k == 7))
                    oo_sb = sm.tile([128, H], bf16, tag="oo", bufs=2,
                                    name=f"oo_{l}")
                    for n in range(2):
                        nc.vector.scalar_tensor_tensor(
                            oo_sb[:, n * 512:(n + 1) * 512], bvb(n), ws[:],
                            oo_ps[n][:], op0=Alu.mult, op1=Alu.add)

                    if l < L - 1:
                        g_full = psW.tile([128, 512], f32, tag="tw",
                                          name=f"g_{l}")
                        g_ps = g_full[:, 0:32]
                        for k in range(8):
                            nc.tensor.matmul(g_ps, tt_sb[:, k, :], a2p(k),
                                             start=(k == 0), stop=(k == 7))
                        g_sb = sm.tile([128, 32], bf16, tag="gsb", bufs=2,
                                       name=f"gsb_{l}")
                        nc.vector.scalar_tensor_tensor(g_sb[:], g0b, ws[:],
                                                       g_ps, op0=Alu.mult,
                                                       op1=Alu.add)
                        for b in range(BPC):
                            nc.tensor.matmul(lg[32 * b:32 * (b + 1), :],
                                             g_sb[32 * b:32 * (b + 1), :],
                                             opwt[32 * b:32 * (b + 1), :],
                                             start=False, stop=(l == L - 2),
                                             tile_position=(32 * b, 32 * b),
                                             skip_group_check=True)
                    st["oo_sb"] = oo_sb

                def emit_s6(l, st):
                    """out + residual adds (updates xn)."""
                    nonlocal opwt_cur, oo_cur
                    opwt_cur, oo_cur = st["opwt"], st["oo_sb"]
                    for cc in range(4):
                        pos = [emit_out_resid(l, b, cc)
                               for b in range(BPC)]
                        for b in range(BPC):
                            emit_add(b, cc, pos[b], l)

                # software-pipelined emission: layer l+1's front fills the
                # engine queues while layer l's residual phase drains
                opwt_cur = oo_cur = None
                states = [None] * L
                states[0] = emit_front(0)
                emit_mid(0, states[0])
                for l in range(1, L):
                    states[l] = emit_front(l)
                    emit_s6(l - 1, states[l - 1])
                    emit_mid(l, states[l])
                emit_s6(L - 1, states[L - 1])
                for b in range(BPC):
                    emit_final(b)

    nc.compile()
    return nc


def _prep_inputs(inputs):
    """Host-side sharding + weight-only preprocessing. in_maps for 8 cores."""
    tokens = np.asarray(inputs["tokens"]).astype(np.int64)
    word_emb = np.ascontiguousarray(np.asarray(inputs["word_emb"], np.float32))
    Wv = np.asarray(inputs["Wv"], np.float32)
    bv = np.asarray(inputs["bv"], np.float32)
    Wk = np.asarray(inputs["Wk"], np.float32)
    bk = np.asarray(inputs["bk"], np.float32)
    Wq = np.asarray(inputs["Wq_op"], np.float32)
    bq = np.asarray(inputs["bq_op"], np.float32)
    ops = np.asarray(inputs["operators"], np.float32)
    Wout = np.asarray(inputs["Wout"], np.float32)

    scale = 1.0 / math.sqrt(H)
    oq = ops @ Wq.T + bq                      # [O, H]
    oqkT = (Wk.T @ oq.T) * scale              # [H, O]
    c = (bk @ oq.T) * scale                   # [O]
    A2 = Wv.T @ oqkT                          # [H, O]
    g0 = bv @ oqkT                            # [O]

    def chunked(a):
        D, N = a.shape
        return np.ascontiguousarray(a.reshape(8, 128, N).transpose(1, 0, 2))

    def chunk_pad32(a16):                     # [H, O] -> [128, 8*32]
        out = np.zeros((128, 8, 32), np.float32)
        out[:, :, :O] = chunked(a16)
        return out.reshape(128, 8 * 32)

    pe = _sinusoidal_pos_emb(S, H)            # [S, H]
    pen = np.ascontiguousarray(
        pe.reshape(4, 128, H).transpose(1, 0, 2)).reshape(128, 4 * H)

    c_strip = np.full((128, 1), -30.0, np.float32)
    ops_strip = np.zeros((128, 512), np.float32)
    for b4 in range(4):
        c_strip[32 * b4:32 * b4 + O, 0] = c
        ops_strip[32 * b4:32 * b4 + O] = ops

    bd = np.zeros((128, 128), np.float32)
    for b4 in range(4):
        bd[32 * b4:32 * (b4 + 1), 32 * b4:32 * (b4 + 1)] = 1.0

    g0p = np.zeros((1, 32), np.float32)
    g0p[0, :O] = g0

    wpk = np.concatenate([
        pen,
        np.tile(bv.reshape(1, H), (128, 1)),
        ops_strip,
        chunk_pad32(oqkT),
        np.eye(128, dtype=np.float32),
        bd,
        chunk_pad32(A2),
        np.tile(g0p, (128, 1)),
    ], axis=1).astype(BF16)
    assert wpk.shape == (128, WC), wpk.shape

    common = {
        "emb": word_emb.astype(BF16),
        "wpk": wpk,
        "cst": c_strip,
        "wvt": chunked(Wv.T.copy()).astype(BF16),
        "wot": chunked(Wout.T.copy()).astype(BF16),
    }

    in_maps = []
    for cid in range(NCORES):
        toks = tokens[cid * BPC:(cid + 1) * BPC]
        idx = np.zeros((128, BPC, S // 16), np.int16)
        for b in range(BPC):
            t16 = toks[b].reshape(S // 16, 16).T.astype(np.int16)
            idx[:, b, :] = np.tile(t16, (8, 1))
        in_maps.append({**common, "tok": idx})
    return in_maps


def kernel(**inputs):
    from concourse.bass_utils import run_bass_kernel_spmd

    if "nc" not in _cache:
        _cache["nc"] = _build_program()
    nc = _cache["nc"]

    in_maps = _prep_inputs(inputs)
    res = run_bass_kernel_spmd(nc, in_maps, list(range(NCORES)))
    outs = []
    for cid in range(NCORES):
        o = res.results[cid]["out"]  # [BPC, 4, 128, OUT] bf16
        outs.append(np.asarray(o, dtype=np.float32).reshape(BPC, S, OUT))
    bout = np.asarray(inputs["bout"], np.float32).reshape(1, 1, OUT)
    return np.concatenate(outs, axis=0) + bout


# revision 19
# speedup vs baseline: 1.2662x; 1.2662x over previous
"""Trainium2 Bass kernel for nn_ComposerModule (dense_transformer), v5.

Data-parallel over batch: 32 batch items -> 8 NeuronCores, 4 per core.

The four per-core batch items are processed TOGETHER in 32-partition strips
of [128, S] tiles (batch b owns partitions 32b..32b+15; rows 32b+16..32b+31
are zero pads).  Per-batch [O=16, S] softmax ops become single [128, S]
ops; thin-M matmuls run 4-way concurrent via tile_position tiling.

The residual stream is kept ONLY in xn ([s, h]) orientation.  Logits are
accumulated INCREMENTALLY in f32 PSUM instead of re-projecting x:
    lg_{l+1} = lg_l + G @ opwt,   G = t @ (Wv^T oqk) + ws x (bv oqk)
xt is materialized only at layer 0 (PE transposes of the gathered
embedding, overlapping the SWDGE gathers) and after the last layer (xbar
transposes feeding the final projection).

v5 vs v4 (both profiled on HW): the embedding PSUM pool is scoped so the
layer loop gets 5 free banks; the out+residual stage is ONE K=32 N=1024
bf16-PSUM matmul per (batch, s-chunk) -- 4-way row-tile concurrent -- and
ONE [128,1024] bf16 2x-mode DVE add (batch 3 goes ACT copy + GPSIMD add
for engine balance); t^T runs on the PE (identity transposes) so the PE
has no idle window there (HAM stays warm).

Algebra: v-projection folded, both softmaxes share one exp:
  w[o,s]  = e[o,s]/rowsum * ops[o,s];  t = w @ x;  oo = t @ Wv^T + ws*bv
  out[s,h] = sum_o e[o,s]/colsum[s] * oo[o,h];  x += out
Pad hygiene: oqkt pad cols are 0 and c pad rows are -30, so
e_pad = exp(-30) ~ 1e-13; ops_strip/A2/g0 pad entries are 0 so
w/ws/t/oo/G/delta-lg pads are exactly 0.
"""
import math

import numpy as np
import ml_dtypes

B, S, H, O, V, OUT, L = 32, 512, 1024, 16, 32000, 1000, 4
NCORES = 8
BPC = B // NCORES
BF16 = ml_dtypes.bfloat16

# packed-weights column offsets (bf16 [128, WC])
_PEN0 = 0              # pe chunked [128, 4*1024]
_BVB0 = 4096           # bv tiled   [128, 1024]
_OPS0 = 5120           # ops strips [128, 512]
_OQK0 = 5632           # oqkT pad   [128, 8*32]
_IDN0 = 5888           # identity   [128, 128]
_BD0 = 6016            # block-diag [128, 128]
_A20 = 6144            # Wv^T@oqkT pad [128, 8*32]
_G00 = 6400            # bv@oqkT pad   [128, 32]
WC = 6432

_cache = {}


def _sinusoidal_pos_emb(seq_len, dim):
    pos = np.arange(seq_len)[:, None].astype(np.float32)
    div = np.exp(np.arange(0, dim, 2).astype(np.float32) * (-math.log(10000.0) / dim))
    pe = np.zeros((seq_len, dim), dtype=np.float32)
    pe[:, 0::2] = np.sin(pos * div)
    pe[:, 1::2] = np.cos(pos * div)
    return pe


def _build_program():
    import concourse.bacc as bacc
    import concourse.bass as bass
    import concourse.tile as tile
    from concourse import mybir

    dt = mybir.dt
    f32, bf16, i16 = dt.float32, dt.bfloat16, dt.int16
    PSUM = bass.MemorySpace.PSUM
    Alu = mybir.AluOpType
    Act = mybir.ActivationFunctionType

    nc = bacc.Bacc("TRN2", target_bir_lowering=False, debug=False, num_devices=NCORES)

    emb_d = nc.declare_dram_parameter("emb", [V, H], bf16, isOutput=False)
    tok_d = nc.declare_dram_parameter("tok", [128, BPC, S // 16], i16, isOutput=False)
    wpk_d = nc.declare_dram_parameter("wpk", [128, WC], bf16, isOutput=False)
    cst_d = nc.declare_dram_parameter("cst", [128, 1], f32, isOutput=False)
    wvt_d = nc.declare_dram_parameter("wvt", [128, 8, H], bf16, isOutput=False)
    wot_d = nc.declare_dram_parameter("wot", [128, 8, OUT], bf16, isOutput=False)
    out_d = nc.declare_dram_parameter("out", [BPC, 4, 128, OUT], bf16, isOutput=True)

    with tile.TileContext(nc) as tc:
        with (
            tc.tile_pool(name="wts", bufs=1) as wp,
            tc.tile_pool(name="xres", bufs=1) as xp,
            tc.tile_pool(name="work", bufs=2) as wk,
            tc.tile_pool(name="sm", bufs=2) as sm,
            tc.tile_pool(name="psG", bufs=1, space=PSUM) as psG,
            tc.tile_pool(name="psW", bufs=2, space=PSUM) as psW,
        ):
            # ---- persistent weights
            wpk = wp.tile([128, WC], bf16)
            c_sb = wp.tile([128, 1], f32)
            wvt = wp.tile([128, 8, H], bf16)
            wot = wp.tile([128, 8, OUT], bf16)
            tokt = wp.tile([128, BPC, S // 16], i16)

            def pen(cc):
                return wpk[:, _PEN0 + cc * H:_PEN0 + (cc + 1) * H]

            def bvb(n):
                return wpk[:, _BVB0 + n * 512:_BVB0 + (n + 1) * 512]

            ops_s = wpk[:, _OPS0:_OPS0 + 512]

            def oqkt(k):
                return wpk[:, _OQK0 + k * 32:_OQK0 + (k + 1) * 32]

            idn = wpk[:, _IDN0:_IDN0 + 128]
            bd = wpk[:, _BD0:_BD0 + 128]

            def a2p(k):
                return wpk[:, _A20 + k * 32:_A20 + (k + 1) * 32]

            g0b = wpk[:, _G00:_G00 + 32]

            # startup loads: tok + packed weights on sync, wvt/wot on scalar
            nc.sync.dma_start(tokt[:], tok_d[:])
            nc.sync.dma_start(wpk[:], wpk_d[:])
            nc.sync.dma_start(c_sb[:], cst_d[:])
            nc.scalar.dma_start(wvt[:, 0:4], wvt_d[:, 0:4])
            nc.scalar.dma_start(wvt[:, 4:8], wvt_d[:, 4:8])

            # ---- residual stream (bf16): xn master; xt only at entry/exit
            xt = [xp.tile([128, 8, S], bf16, name=f"xt{b}") for b in range(BPC)]
            xn = [xp.tile([128, 4, H], bf16, name=f"xn{b}") for b in range(BPC)]

            # persistent f32 logits accumulator [strip, s]
            lg = psG.tile([128, S], f32, name="lg")

            # ---- embedding: SWDGE gather -> xn; +pe; PE-transpose -> xt0;
            # layer-0 logits emitted per batch (keeps the PE FIFO flowing)
            with tc.tile_pool(name="psT", bufs=2, space=PSUM) as psT:
                for b in range(BPC):
                    for hh in range(2):
                        nc.gpsimd.dma_gather(
                            out_ap=xn[b][:, hh * 2:(hh + 1) * 2, :],
                            in_ap=emb_d[:],
                            idxs_ap=tokt[:, b, hh * 16:(hh + 1) * 16],
                            num_idxs=S // 2, num_idxs_reg=S // 2, elem_size=H,
                            transpose=False)
                for b in range(BPC):
                    for cc in range(4):
                        nc.vector.tensor_tensor(xn[b][:, cc, :],
                                                xn[b][:, cc, :],
                                                pen(cc), op=Alu.add)
                    for k in range(8):
                        ttp = psT.tile([128, 4, 128], bf16, tag="tr", bufs=2,
                                       name=f"ept_{b}_{k}")
                        for cc in range(4):
                            nc.tensor.transpose(
                                ttp[:, cc, :],
                                xn[b][:, cc, k * 128:(k + 1) * 128], idn)
                        if k % 2 == 0:
                            nc.vector.tensor_copy(xt[b][:, k, :], ttp[:])
                        else:
                            nc.scalar.copy(xt[b][:, k, :], ttp[:])
                    for k in range(8):
                        nc.tensor.matmul(lg[32 * b:32 * (b + 1), :],
                                         oqkt(k), xt[b][:, k, :],
                                         start=(k == 0), stop=False,
                                         tile_position=(0, 32 * b),
                                         skip_group_check=True)

            # final-projection weights (needed only after the last layer)
            nc.scalar.dma_start(wot[:, 0:4], wot_d[:, 0:4])
            nc.scalar.dma_start(wot[:, 4:8], wot_d[:, 4:8])

            with tc.tile_pool(name="psO", bufs=5, space=PSUM) as psO:

                def emit_final(b):
                    # out[s, :] = x @ Wout^T  (bout added on host)
                    for cc in range(4):
                        nc.sync.dma_start(
                            xt[b][:, :, cc * 128:(cc + 1) * 128],
                            xn[b][:, cc, :], transpose=True)
                    for mg in range(4):
                        osb = wk.tile([128, OUT], bf16, tag="osb", bufs=3)
                        pA = psO.tile([128, 512], f32, tag="rs", bufs=5,
                                      name=f"fA_{b}_{mg}")
                        pB = psO.tile([128, 512], f32, tag="rs", bufs=5,
                                      name=f"fB_{b}_{mg}")
                        for k in range(8):
                            nc.tensor.matmul(
                                pA[:], xt[b][:, k, mg * 128:(mg + 1) * 128],
                                wot[:, k, 0:512], start=(k == 0),
                                stop=(k == 7))
                            nc.tensor.matmul(
                                pB[:, 0:OUT - 512],
                                xt[b][:, k, mg * 128:(mg + 1) * 128],
                                wot[:, k, 512:OUT], start=(k == 0),
                                stop=(k == 7))
                        nc.vector.tensor_copy(osb[:, 0:512], pA[:])
                        nc.scalar.copy(osb[:, 512:OUT], pB[:, 0:OUT - 512])
                        nc.sync.dma_start(out_d[b, mg], osb[:])

                def emit_out_resid(l, b, cc):
                    # n=0 half: plain matmul, drained by a DVE psum add
                    po = psO.tile([128, 512], f32, tag="rs", bufs=5,
                                  name=f"o_{l}_{cc}_{b}")
                    nc.tensor.matmul(
                        po[:],
                        opwt_cur[32 * b:32 * (b + 1),
                                 cc * 128:(cc + 1) * 128],
                        oo_cur[32 * b:32 * (b + 1), 0:512],
                        start=True, stop=True, tile_position=(32 * b, 0))
                    # n=1 half: b 0/1 plain (ACT copy + GPSIMD add), b 2/3
                    # PE identity-accumulate (ACT copy)
                    q = psO.tile([128, 512], f32, tag="rs", bufs=5,
                                 name=f"q_{l}_{cc}_{b}")
                    if b >= 2:
                        nc.tensor.matmul(q[:], idn, xn[b][:, cc, 512:1024],
                                         start=True, stop=False)
                    nc.tensor.matmul(
                        q[:],
                        opwt_cur[32 * b:32 * (b + 1),
                                 cc * 128:(cc + 1) * 128],
                        oo_cur[32 * b:32 * (b + 1), 512:1024],
                        start=(b < 2), stop=True,
                        skip_group_check=True, tile_position=(32 * b, 0))
                    return po, q

                def emit_add(b, cc, poq, l):
                    po, q = poq
                    nc.vector.tensor_tensor(xn[b][:, cc, 0:512],
                                            xn[b][:, cc, 0:512],
                                            po[:], op=Alu.add)
                    if b < 2:
                        rtmp = sm.tile([128, 512], bf16, tag="rtmp", bufs=2,
                                       name=f"rt_{l}_{cc}_{b}")
                        nc.scalar.copy(rtmp[:], q[:])
                        nc.gpsimd.tensor_tensor(xn[b][:, cc, 512:1024],
                                                xn[b][:, cc, 512:1024],
                                                rtmp[:], op=Alu.add)
                    else:
                        nc.scalar.copy(xn[b][:, cc, 512:1024], q[:])

                # ---- layer stages (stage-major emission, all batches)
                def emit_front(l):
                    """S2-S4a: exp, colsum/reciprocals, w, opwt, w^T DMA.
                    Depends only on lg (deltas from layer l-1), not on xn."""
                    e_all = sm.tile([128, S], bf16, tag="e", bufs=2,
                                    name=f"e_{l}")
                    rs = sm.tile([128, 1], f32, tag="rs", bufs=2)
                    nc.scalar.activation(e_all[:], lg[:], Act.Exp,
                                         bias=c_sb[:], accum_out=rs[:])

                    cs_ps = psW.tile([128, S], f32, tag="tw", name=f"cs_{l}")
                    nc.tensor.matmul(cs_ps[:], bd[:], e_all[:], start=True,
                                     stop=True)
                    rcb = sm.tile([128, S], f32, tag="rcb", bufs=2,
                                  name=f"rcb_{l}")
                    nc.vector.reciprocal_approx_fast(rcb[:], cs_ps[:])

                    rcs = sm.tile([128, 1], f32, tag="rcs", bufs=2)
                    nc.vector.reciprocal(rcs[:], rs[:])
                    w_all = sm.tile([128, S], bf16, tag="w", bufs=2,
                                    name=f"w_{l}")
                    ws = sm.tile([128, 1], f32, tag="ws", bufs=2)
                    nc.vector.scalar_tensor_tensor(w_all[:], e_all[:],
                                                   rcs[:], ops_s,
                                                   op0=Alu.mult,
                                                   op1=Alu.mult,
                                                   accum_out=ws[:])

                    opwt = sm.tile([128, S], bf16, tag="opwt", bufs=2,
                                   name=f"opwt_{l}")
                    nc.vector.tensor_tensor(opwt[:], e_all[:], rcb[:],
                                            op=Alu.mult)

                    wt_sb = sm.tile([128, 4, 128], bf16, tag="wt", bufs=2,
                                    name=f"wt_{l}")
                    nc.sync.dma_start(wt_sb[:], w_all[:], transpose=True)
                    return {"opwt": opwt, "ws": ws, "wt_sb": wt_sb}

                def emit_mid(l, st):
                    """S4b-S5b: t, t^T, oo, and the lg delta for layer l+1.
                    Needs xn (layer l-1's residual complete)."""
                    wt_sb, ws, opwt = st["wt_sb"], st["ws"], st["opwt"]
                    t_ps = [psW.tile([128, 512], f32, tag="tw",
                                     name=f"t_{l}_{n}") for n in range(2)]
                    for cc in range(4):
                        for n in range(2):
                            for b in range(BPC):
                                nc.tensor.matmul(
                                    t_ps[n][32 * b:32 * (b + 1), :],
                                    wt_sb[:, cc, 32 * b:32 * (b + 1)],
                                    xn[b][:, cc, n * 512:(n + 1) * 512],
                                    start=(cc == 0), stop=(cc == 3),
                                    tile_position=(0, 32 * b))
                    t_sb = sm.tile([128, H], bf16, tag="tsb", bufs=2,
                                   name=f"t_{l}")
                    nc.scalar.copy(t_sb[:, 0:512], t_ps[0][:])
                    nc.scalar.copy(t_sb[:, 512:], t_ps[1][:])

                    tt_sb = sm.tile([128, 8, 128], bf16, tag="tt", bufs=2,
                                    name=f"tt_{l}")
                    for g in range(2):
                        trp = psW.tile([128, 4, 128], bf16, tag="tw",
                                       name=f"tr_{l}_{g}")
                        for k in range(4):
                            nc.tensor.transpose(
                                trp[:, k, :],
                                t_sb[:, (g * 4 + k) * 128:
                                     (g * 4 + k + 1) * 128], idn)
                        nc.vector.tensor_copy(tt_sb[:, g * 4:(g + 1) * 4],
                                              trp[:])
                    oo_ps = [psW.tile([128, 512], f32, tag="tw",
                                      name=f"oo_{l}_{n}") for n in range(2)]
                    for n in range(2):
                        for k in range(8):
                            nc.tensor.matmul(oo_ps[n][:], tt_sb[:, k, :],
                                             wvt[:, k,
                                                 n * 512:(n + 1) * 512],
                                             start=(k == 0), stop=(k == 7))
                    oo_sb = sm.tile([128, H], bf16, tag="oo", bufs=2,
                                    name=f"oo_{l}")
                    for n in range(2):
                        nc.vector.scalar_tensor_tensor(
                            oo_sb[:, n * 512:(n + 1) * 512], bvb(n), ws[:],
                            oo_ps[n][:], op0=Alu.mult, op1=Alu.add)

                    if l < L - 1:
                        g_full = psW.tile([128, 512], f32, tag="tw",
                                          name=f"g_{l}")
                        g_ps = g_full[:, 0:32]
                        for k in range(8):
                            nc.tensor.matmul(g_ps, tt_sb[:, k, :], a2p(k),
                                             start=(k == 0), stop=(k == 7))
                        g_sb = sm.tile([128, 32], bf16, tag="gsb", bufs=2,
                                       name=f"gsb_{l}")
                        nc.vector.scalar_tensor_tensor(g_sb[:], g0b, ws[:],
                                                       g_ps, op0=Alu.mult,
                                                       op1=Alu.add)
                        for b in range(BPC):
                            nc.tensor.matmul(lg[32 * b:32 * (b + 1), :],
                                             g_sb[32 * b:32 * (b + 1), :],
                                             opwt[32 * b:32 * (b + 1), :],
                                             start=False, stop=(l == L - 2),
                                             tile_position=(32 * b, 32 * b),
                                             skip_group_check=True)
                    st["oo_sb"] = oo_sb

                def emit_s6(l, st):
                    """out + residual adds (updates xn)."""
                    nonlocal opwt_cur, oo_cur
                    opwt_cur, oo_cur = st["opwt"], st["oo_sb"]
                    for cc in range(4):
                        pos = [emit_out_resid(l, b, cc)
                               for b in range(BPC)]
                        for b in range(BPC):
                            emit_add(b, cc, pos[b], l)

                # straight emission (measured faster than software-
                # pipelining the front of layer l+1 into layer l's S6)
                opwt_cur = oo_cur = None
                for l in range(L):
                    st = emit_front(l)
                    emit_mid(l, st)
                    emit_s6(l, st)
                for b in range(BPC):
                    emit_final(b)

    nc.compile()
    return nc


def _prep_inputs(inputs):
    """Host-side sharding + weight-only preprocessing. in_maps for 8 cores."""
    tokens = np.asarray(inputs["tokens"]).astype(np.int64)
    word_emb = np.ascontiguousarray(np.asarray(inputs["word_emb"], np.float32))
    Wv = np.asarray(inputs["Wv"], np.float32)
    bv = np.asarray(inputs["bv"], np.float32)
    Wk = np.asarray(inputs["Wk"], np.float32)
    bk = np.asarray(inputs["bk"], np.float32)
    Wq = np.asarray(inputs["Wq_op"], np.float32)
    bq = np.asarray(inputs["bq_op"], np.float32)
    ops = np.asarray(inputs["operators"], np.float32)
    Wout = np.asarray(inputs["Wout"], np.float32)

    scale = 1.0 / math.sqrt(H)
    oq = ops @ Wq.T + bq                      # [O, H]
    oqkT = (Wk.T @ oq.T) * scale              # [H, O]
    c = (bk @ oq.T) * scale                   # [O]
    A2 = Wv.T @ oqkT                          # [H, O]
    g0 = bv @ oqkT                            # [O]

    def chunked(a):
        D, N = a.shape
        return np.ascontiguousarray(a.reshape(8, 128, N).transpose(1, 0, 2))

    def chunk_pad32(a16):                     # [H, O] -> [128, 8*32]
        out = np.zeros((128, 8, 32), np.float32)
        out[:, :, :O] = chunked(a16)
        return out.reshape(128, 8 * 32)

    pe = _sinusoidal_pos_emb(S, H)            # [S, H]
    pen = np.ascontiguousarray(
        pe.reshape(4, 128, H).transpose(1, 0, 2)).reshape(128, 4 * H)

    c_strip = np.full((128, 1), -30.0, np.float32)
    ops_strip = np.zeros((128, 512), np.float32)
    for b4 in range(4):
        c_strip[32 * b4:32 * b4 + O, 0] = c
        ops_strip[32 * b4:32 * b4 + O] = ops

    bd = np.zeros((128, 128), np.float32)
    for b4 in range(4):
        bd[32 * b4:32 * (b4 + 1), 32 * b4:32 * (b4 + 1)] = 1.0

    g0p = np.zeros((1, 32), np.float32)
    g0p[0, :O] = g0

    wpk = np.concatenate([
        pen,
        np.tile(bv.reshape(1, H), (128, 1)),
        ops_strip,
        chunk_pad32(oqkT),
        np.eye(128, dtype=np.float32),
        bd,
        chunk_pad32(A2),
        np.tile(g0p, (128, 1)),
    ], axis=1).astype(BF16)
    assert wpk.shape == (128, WC), wpk.shape

    common = {
        "emb": word_emb.astype(BF16),
        "wpk": wpk,
        "cst": c_strip,
        "wvt": chunked(Wv.T.copy()).astype(BF16),
        "wot": chunked(Wout.T.copy()).astype(BF16),
    }

    in_maps = []
    for cid in range(NCORES):
        toks = tokens[cid * BPC:(cid + 1) * BPC]
        idx = np.zeros((128, BPC, S // 16), np.int16)
        for b in range(BPC):
            t16 = toks[b].reshape(S // 16, 16).T.astype(np.int16)
            idx[:, b, :] = np.tile(t16, (8, 1))
        in_maps.append({**common, "tok": idx})
    return in_maps


def kernel(**inputs):
    from concourse.bass_utils import run_bass_kernel_spmd

    if "nc" not in _cache:
        _cache["nc"] = _build_program()
    nc = _cache["nc"]

    in_maps = _prep_inputs(inputs)
    res = run_bass_kernel_spmd(nc, in_maps, list(range(NCORES)))
    outs = []
    for cid in range(NCORES):
        o = res.results[cid]["out"]  # [BPC, 4, 128, OUT] bf16
        outs.append(np.asarray(o, dtype=np.float32).reshape(BPC, S, OUT))
    bout = np.asarray(inputs["bout"], np.float32).reshape(1, 1, OUT)
    return np.concatenate(outs, axis=0) + bout
